# revision 30
# baseline (speedup 1.0000x reference)
"""ODE-RNN encoder (GRU-ODE scan) Trainium2 Bass kernel.

Strategy (data-parallel over trajectories):
  - 4096 trajectories sharded 512/core over 8 NeuronCores; all weights
    replicated. The T=128 time scan runs locally per core, no cross-core
    communication. Host gathers the per-core z0 outputs at the end.
  - On-chip layout is feature-on-partition, batch-on-free-dim, bf16
    everywhere except PSUM accumulation. Each core's 512-batch is split
    into 2 dephased chunks of 256 so the serial per-step dependency chain
    of one chunk hides under engine work of the other.
  - The wall clock is 128 x the single-step chain latency, so the design
    minimizes chain hops: 11 matmuls per chunk-step (no ODE-correction
    matmuls, no -I matmul), gates read the post-ODE state written in
    place by one DVE op, and the blend top half is
      y' = (y_ode - g*y_ode) + g*ns_t
    with the (1-g)*y_ode part computed off-chain. abs(ns_b) runs on DVE
    (max(x,-x)) to keep the ACT engine at 6 ops/chunk-step.
  - PSUM start=True zeroes the ENTIRE 2 KiB bank (hardware zero region),
    so each bank runs ONE accumulation group per step: first matmul
    carries start, last carries stop, middles accumulate. Banks per
    chunk: A1=[ode hidden|ode out], A2=[gate1 u|r], B1=[gate2 u|r],
    B2=[ns hidden|ns out]. Cross-step zeroing safety is by engine
    queue order (see stage list comments).

kernel(**inputs) takes the full unsharded numpy inputs (as produced by the
reference setup) and returns (z0_mu, z0_std), each (1, 4096, 64) float32.
"""

import os
import sys

import numpy as np

N_TRAJ = 4096
T = 128
LAT = 64
NDATA = 64
INP = 2 * NDATA
NGRU = 100
NODE = 100
TZ = 100
NCORES = 8
B = N_TRAJ // NCORES          # 512 per core
CH = 2                        # chunks per core
BC = B // CH                  # 256 batch per chunk

_cache = {}


def _build(dts, use_bias):
    import concourse.bass as bass
    import concourse.tile as tile
    from concourse import bacc, mybir

    uniq = list(dict.fromkeys(dts))
    dt_idx = [uniq.index(d) for d in dts]
    n_dt = len(uniq)

    f32 = mybir.dt.float32
    bf16 = mybir.dt.bfloat16
    ACT = mybir.ActivationFunctionType
    ALU = mybir.AluOpType

    nc = bacc.Bacc("TRN2", target_bir_lowering=False, debug=False,
                   num_devices=NCORES)

    # ---- DRAM I/O ----
    xT_d = nc.dram_tensor("xT", [T, INP, B], bf16, kind="ExternalInput")
    wug1_d = nc.dram_tensor("wug1", [2 * LAT + INP, NGRU], bf16, kind="ExternalInput")
    wrg1_d = nc.dram_tensor("wrg1", [2 * LAT + INP, NGRU], bf16, kind="ExternalInput")
    wns1_d = nc.dram_tensor("wns1", [2 * LAT + INP, NGRU], bf16, kind="ExternalInput")
    wug2_d = nc.dram_tensor("wug2nd", [NGRU, 2 * LAT], bf16, kind="ExternalInput")
    wrg2_d = nc.dram_tensor("wrg2d", [NGRU, 2 * LAT], bf16, kind="ExternalInput")
    wns2_d = nc.dram_tensor("wns2", [NGRU, 2 * LAT], bf16, kind="ExternalInput")
    wode1_d = nc.dram_tensor("wode1", [LAT, NODE], bf16, kind="ExternalInput")
    wode2_d = nc.dram_tensor("wode2", [NODE, LAT], bf16, kind="ExternalInput")
    wfug_d = nc.dram_tensor("wfuse_ug", [n_dt, NODE, NGRU], bf16, kind="ExternalInput")
    wfrg_d = nc.dram_tensor("wfuse_rg", [n_dt, NODE, NGRU], bf16, kind="ExternalInput")
    wtz1_d = nc.dram_tensor("wtz1", [2 * LAT, TZ], bf16, kind="ExternalInput")
    wtz2_d = nc.dram_tensor("wtz2", [TZ, 2 * LAT], bf16, kind="ExternalInput")
    if use_bias:
        bode1_d = nc.dram_tensor("bode1", [NODE, 1], f32, kind="ExternalInput")
        bug1_d = nc.dram_tensor("bug1", [NGRU, 1], f32, kind="ExternalInput")
        brg1_d = nc.dram_tensor("brg1", [NGRU, 1], f32, kind="ExternalInput")
        bns1_d = nc.dram_tensor("bns1", [NGRU, 1], f32, kind="ExternalInput")
        bns2b_d = nc.dram_tensor("bns2b", [LAT, 1], f32, kind="ExternalInput")
        btz1_d = nc.dram_tensor("btz1", [TZ, 1], f32, kind="ExternalInput")
        btz2t_d = nc.dram_tensor("btz2t", [LAT, 1], f32, kind="ExternalInput")
        btz2b_d = nc.dram_tensor("btz2b", [LAT, 1], f32, kind="ExternalInput")
        # row-vector biases (K=1 matmul accumulate): [1, M]
        bug2_d = nc.dram_tensor("bug2ndr", [1, 2 * LAT], bf16, kind="ExternalInput")
        brg2_d = nc.dram_tensor("brg2dr", [1, 2 * LAT], bf16, kind="ExternalInput")
        bns2t_d = nc.dram_tensor("bns2tr", [1, LAT], bf16, kind="ExternalInput")
        bode2_d = nc.dram_tensor("bode2r", [1, LAT], bf16, kind="ExternalInput")
        ones_d = nc.dram_tensor("ones1", [1, BC], bf16, kind="ExternalInput")
    zeros_d = nc.dram_tensor("zeros0", [2 * LAT, B], bf16, kind="ExternalInput")
    zout_d = nc.dram_tensor("zout", [2 * LAT, B], f32, kind="ExternalOutput")

    with tile.TileContext(nc) as tc:
        with (
            tc.tile_pool(name="const", bufs=1) as cpool,
            tc.tile_pool(name="state", bufs=1) as spool,
            tc.tile_pool(name="xin", bufs=3) as xpool,
            tc.tile_pool(name="mdup", bufs=2) as mpool,
            tc.tile_pool(name="tmp0", bufs=2) as tpool0,
            tc.tile_pool(name="tmp1", bufs=2) as tpool1,
            tc.tile_pool(name="bkA10", bufs=1, space="PSUM") as bA10,
            tc.tile_pool(name="bkA20", bufs=1, space="PSUM") as bA20,
            tc.tile_pool(name="bkB10", bufs=1, space="PSUM") as bB10,
            tc.tile_pool(name="bkB20", bufs=1, space="PSUM") as bB20,
            tc.tile_pool(name="bkA11", bufs=1, space="PSUM") as bA11,
            tc.tile_pool(name="bkA21", bufs=1, space="PSUM") as bA21,
            tc.tile_pool(name="bkB11", bufs=1, space="PSUM") as bB11,
            tc.tile_pool(name="bkB21", bufs=1, space="PSUM") as bB21,
        ):
            tpool = [tpool0, tpool1]
            bA1p = [bA10, bA11]
            bA2p = [bA20, bA21]
            bB1p = [bB10, bB11]
            bB2p = [bB20, bB21]

            # ---- load constants ----
            def cload(shape, src_ap, tag, dt_=None):
                t = cpool.tile(shape, dt_ or bf16, tag=tag, name=tag)
                nc.sync.dma_start(t[:, :], src_ap)
                return t

            wug1a = cload([INP, NGRU], wug1_d[0:INP, :], "wug1a")
            wug1b = cload([INP, NGRU], wug1_d[INP:2 * LAT + INP, :], "wug1b")
            wrg1a = cload([INP, NGRU], wrg1_d[0:INP, :], "wrg1a")
            wrg1b = cload([INP, NGRU], wrg1_d[INP:2 * LAT + INP, :], "wrg1b")
            wns1a = cload([INP, NGRU], wns1_d[0:INP, :], "wns1a")
            wns1b = cload([INP, NGRU], wns1_d[INP:2 * LAT + INP, :], "wns1b")
            # rows 0:128 of w*1 multiply [y;s] (=128 rows), rows 128:256
            # multiply x (=128 rows); INP == 2*LAT == 128 here.
            wug2 = cload([NGRU, 2 * LAT], wug2_d[:, :], "wug2")
            wrg2 = cload([NGRU, 2 * LAT], wrg2_d[:, :], "wrg2")
            wns2 = cload([NGRU, 2 * LAT], wns2_d[:, :], "wns2")
            wode1 = cload([LAT, NODE], wode1_d[:, :], "wode1")
            wode2 = cload([NODE, LAT], wode2_d[:, :], "wode2")
            wfug = [cload([NODE, NGRU], wfug_d[i], f"wfug{i}")
                    for i in range(n_dt)]
            wfrg = [cload([NODE, NGRU], wfrg_d[i], f"wfrg{i}")
                    for i in range(n_dt)]
            wtz1 = cload([2 * LAT, TZ], wtz1_d[:, :], "wtz1")
            wtz2 = cload([TZ, 2 * LAT], wtz2_d[:, :], "wtz2")
            if use_bias:
                bode1 = cload([NODE, 1], bode1_d[:, :], "bode1", f32)
                bug1 = cload([NGRU, 1], bug1_d[:, :], "bug1", f32)
                brg1 = cload([NGRU, 1], brg1_d[:, :], "brg1", f32)
                bns1 = cload([NGRU, 1], bns1_d[:, :], "bns1", f32)
                btz1 = cload([TZ, 1], btz1_d[:, :], "btz1", f32)
                btz2t = cload([LAT, 1], btz2t_d[:, :], "btz2t", f32)
                bns2b = cpool.tile([2 * LAT, 1], f32, tag="bns2b", name="bns2b")
                nc.sync.dma_start(bns2b[LAT:2 * LAT, :], bns2b_d[:, :])
                btz2b = cpool.tile([2 * LAT, 1], f32, tag="btz2b", name="btz2b")
                nc.sync.dma_start(btz2b[LAT:2 * LAT, :], btz2b_d[:, :])
                bug2r = cload([1, 2 * LAT], bug2_d[:, :], "bug2r")
                brg2r = cload([1, 2 * LAT], brg2_d[:, :], "brg2r")
                bns2tr = cload([1, LAT], bns2t_d[:, :], "bns2tr")
                bode2r = cload([1, LAT], bode2_d[:, :], "bode2r")
                ones = cpool.tile([1, BC], bf16, tag="ones", name="ones")
                nc.sync.dma_start(ones[:, :], ones_d[:, :])

            def b_act(t):
                return t[:, :] if use_bias else 0.0

            # ---- state tiles (ping-pong per chunk) ----
            S = [[spool.tile([2 * LAT, BC], bf16, tag=f"s{c}_{p}",
                             name=f"s{c}_{p}")
                  for p in range(2)] for c in range(CH)]
            for c in range(CH):
                nc.sync.dma_start(S[c][0][:, :],
                                  zeros_d[:, c * BC:(c + 1) * BC])

            # ---- PSUM banks: 4 per chunk, 8 total (each [128, 2*BC] f32
            # = 2 KiB/partition = exactly one bank) ----
            bankA1 = [bA1p[c].tile([128, 2 * BC], f32, tag="bA1",
                                   name=f"bA1_{c}") for c in range(CH)]
            bankA2 = [bA2p[c].tile([128, 2 * BC], f32, tag="bA2",
                                   name=f"bA2_{c}") for c in range(CH)]
            bankB1 = [bB1p[c].tile([128, 2 * BC], f32, tag="bB1",
                                   name=f"bB1_{c}") for c in range(CH)]
            bankB2 = [bB2p[c].tile([128, 2 * BC], f32, tag="bB2",
                                   name=f"bB2_{c}") for c in range(CH)]

            def new_ctx(c, t):
                return dict(cs=slice(c * BC, (c + 1) * BC),
                            Sc=S[c][t % 2], Sn=S[c][(t + 1) % 2],
                            tp=tpool[c], t=t,
                            A1=bankA1[c], A2=bankA2[c],
                            B1=bankB1[c], B2=bankB2[c])

            # --- stages, emission order == chain order ---
            def s_ode1(c, d, xt, m2):
                d['oh'] = d['A1'][0:NODE, 0:BC]
                d['ode1'] = nc.tensor.matmul(d['oh'], wode1[:, :],
                                             d['Sc'][0:LAT, :],
                                             start=True, stop=False)

            def s_g1x_u(c, d, xt, m2):
                d['g1'] = d['A2'][0:NGRU, 0:2 * BC]
                nc.tensor.matmul(d['g1'][:, 0:BC], wug1b[:, :],
                                 xt[:, d['cs']], start=True, stop=False)

            def s_g1x_r(c, d, xt, m2):
                nc.tensor.matmul(d['g1'][:, BC:2 * BC], wrg1b[:, :],
                                 xt[:, d['cs']], start=False, stop=False,
                                 skip_group_check=True)

            def s_tanh_ode(c, d, xt, m2):
                d['h_ode'] = d['tp'].tile([NODE, BC], bf16, tag="h_ode",
                                          name=f"ho{c}")
                nc.scalar.activation(d['h_ode'][:, :], d['oh'], ACT.Tanh,
                                     bias=b_act(bode1) if use_bias else 0.0)

            def s_ode2(c, d, xt, m2):
                d['yo'] = d['A1'][0:LAT, BC:2 * BC]
                nc.tensor.matmul(d['yo'], wode2[:, :], d['h_ode'][:, :],
                                 start=False, stop=not use_bias,
                                 skip_group_check=True)
                if use_bias:
                    nc.tensor.matmul(d['yo'], bode2r[:, :], ones[:, :],
                                     start=False, stop=True,
                                     skip_group_check=True)

            def s_yode(c, d, xt, m2):
                # in-place: Sc[0:LAT] <- y + dt * ode_out
                nc.vector.scalar_tensor_tensor(
                    d['Sc'][0:LAT, :], d['yo'], float(dts[d['t']]),
                    d['Sc'][0:LAT, :], op0=ALU.mult, op1=ALU.add)

            # gate-1 state matmuls read the PRE-ODE state (emitted before
            # yode's in-place write); the fused dt*(ode_w2 @ W_g1[:64])
            # correction matmuls bring them to the post-ODE value without
            # waiting on the yode DVE hop.
            def s_g1s_r(c, d, xt, m2):
                nc.tensor.matmul(d['g1'][:, BC:2 * BC], wrg1a[:, :],
                                 d['Sc'][:, :], start=False, stop=False,
                                 skip_group_check=True)

            def s_g1c_r(c, d, xt, m2):
                nc.tensor.matmul(d['g1'][:, BC:2 * BC],
                                 wfrg[dt_idx[d['t']]][:, :],
                                 d['h_ode'][:, :], start=False, stop=False,
                                 skip_group_check=True)

            def s_g1c_u(c, d, xt, m2):
                nc.tensor.matmul(d['g1'][:, 0:BC],
                                 wfug[dt_idx[d['t']]][:, :],
                                 d['h_ode'][:, :], start=False, stop=True,
                                 skip_group_check=True)

            def s_tanh_r(c, d, xt, m2):
                d['h_g'] = d['tp'].tile([NGRU, 2 * BC], bf16, tag="h_g",
                                        name=f"hg{c}")
                nc.scalar.activation(d['h_g'][:, BC:2 * BC],
                                     d['g1'][:, BC:2 * BC], ACT.Tanh,
                                     bias=b_act(brg1) if use_bias else 0.0)

            def s_ns1x(c, d, xt, m2):
                d['n1'] = d['B2'][0:NGRU, 0:BC]
                nc.tensor.matmul(d['n1'], wns1b[:, :],
                                 xt[:, d['cs']], start=True, stop=False)

            def s_g1s_u(c, d, xt, m2):
                nc.tensor.matmul(d['g1'][:, 0:BC], wug1a[:, :],
                                 d['Sc'][:, :], start=False, stop=False,
                                 skip_group_check=True)

            def s_rg2(c, d, xt, m2):
                d['g2'] = d['B1'][0:2 * LAT, 0:2 * BC]
                nc.tensor.matmul(d['g2'][:, BC:2 * BC], wrg2[:, :],
                                 d['h_g'][:, BC:2 * BC],
                                 start=True, stop=False)
                if use_bias:
                    nc.tensor.matmul(d['g2'][:, BC:2 * BC], brg2r[:, :],
                                     ones[:, :], start=False, stop=False,
                                     skip_group_check=True)

            def s_sig_r(c, d, xt, m2):
                d['vr'] = d['tp'].tile([2 * LAT, 2 * BC], bf16, tag="vr",
                                       name=f"vr{c}")
                d['sig'] = nc.scalar.activation(d['vr'][:, BC:2 * BC],
                                                d['g2'][:, BC:2 * BC],
                                                ACT.Sigmoid)

            def s_tanh_u(c, d, xt, m2):
                nc.scalar.activation(d['h_g'][:, 0:BC], d['g1'][:, 0:BC],
                                     ACT.Tanh,
                                     bias=b_act(bug1) if use_bias else 0.0)

            def s_ryc(c, d, xt, m2):
                d['ryc'] = d['tp'].tile([2 * LAT, BC], bf16, tag="ryc",
                                        name=f"ryc{c}")
                nc.vector.tensor_mul(d['ryc'][:, :], d['vr'][:, BC:2 * BC],
                                     d['Sc'][:, :])

            def s_ug2(c, d, xt, m2):
                nc.tensor.matmul(d['g2'][:, 0:BC], wug2[:, :],
                                 d['h_g'][:, 0:BC],
                                 start=False, stop=not use_bias,
                                 skip_group_check=True)
                if use_bias:
                    nc.tensor.matmul(d['g2'][:, 0:BC], bug2r[:, :],
                                     ones[:, :], start=False, stop=True,
                                     skip_group_check=True)

            def s_ns1s(c, d, xt, m2):
                nc.tensor.matmul(d['n1'], wns1a[:, :],
                                 d['ryc'][:, :], start=False, stop=False,
                                 skip_group_check=True)

            def s_sig_v(c, d, xt, m2):
                nc.scalar.activation(d['vr'][:, 0:BC], d['g2'][:, 0:BC],
                                     ACT.Sigmoid)

            def s_gm(c, d, xt, m2):
                d['g'] = d['tp'].tile([2 * LAT, BC], bf16, tag="g",
                                      name=f"g{c}")
                nc.vector.tensor_mul(d['g'][:, :], m2[:, d['cs']],
                                     d['vr'][:, 0:BC])

            def s_gt1(c, d, xt, m2):
                # t1 = g_top * y_ode (feeds add_top, keep on fast DVE)
                d['t1'] = d['tp'].tile([LAT, BC], bf16, tag="t1",
                                       name=f"t1{c}")
                nc.vector.tensor_mul(d['t1'][:, :], d['g'][0:LAT, :],
                                     d['Sc'][0:LAT, :])

            def s_uy(c, d, xt, m2):
                # uy = (1 - g_top) * y_ode
                d['uy'] = d['tp'].tile([LAT, BC], bf16, tag="uy",
                                       name=f"uy{c}")
                nc.vector.tensor_sub(d['uy'][:, :], d['Sc'][0:LAT, :],
                                     d['t1'][:, :])

            def s_tanh_ns(c, d, xt, m2):
                d['h_n'] = d['tp'].tile([NGRU, BC], bf16, tag="h_n",
                                        name=f"hn{c}")
                nc.scalar.activation(d['h_n'][:, :], d['n1'], ACT.Tanh,
                                     bias=b_act(bns1) if use_bias else 0.0)

            def s_ns2(c, d, xt, m2):
                d['n2'] = d['B2'][0:2 * LAT, BC:2 * BC]
                nc.tensor.matmul(d['n2'], wns2[:, :], d['h_n'][:, :],
                                 start=False, stop=not use_bias,
                                 skip_group_check=True)
                if use_bias:
                    nc.tensor.matmul(d['n2'][0:LAT, :], bns2tr[:, :],
                                     ones[:, :], start=False, stop=True,
                                     skip_group_check=True)

            def s_abs(c, d, xt, m2):
                # |ns_b| on DVE: max(x, -x); ACT path only when a bias
                # must be added before the abs.
                d['absb'] = d['tp'].tile([2 * LAT, BC], bf16, tag="absb",
                                         name=f"ab{c}")
                nc.scalar.activation(d['absb'][LAT:2 * LAT, :],
                                     d['n2'][LAT:2 * LAT, :], ACT.Abs,
                                     bias=bns2b[LAT:2 * LAT, :] if use_bias
                                     else 0.0)

            def s_gtq_top(c, d, xt, m2):
                d['gtq'] = d['tp'].tile([2 * LAT, BC], bf16, tag="gtq",
                                        name=f"gtq{c}")
                nc.vector.tensor_mul(d['gtq'][0:LAT, :], d['g'][0:LAT, :],
                                     d['n2'][0:LAT, :])

            def s_add_top(c, d, xt, m2):
                nc.vector.tensor_add(d['Sn'][0:LAT, :], d['uy'][:, :],
                                     d['gtq'][0:LAT, :])

            # bottom (std) half of the blend is pure slack: its result is
            # first needed by g1s of the NEXT step, ~a full half-period
            # later, so it runs on the otherwise-idle GPSIMD engine.
            def s_qb(c, d, xt, m2):
                d['qb'] = d['tp'].tile([2 * LAT, BC], bf16, tag="qb",
                                       name=f"qb{c}")
                nc.gpsimd.tensor_sub(d['qb'][LAT:2 * LAT, :],
                                     d['absb'][LAT:2 * LAT, :],
                                     d['Sc'][LAT:2 * LAT, :])

            def s_gtq_bot(c, d, xt, m2):
                nc.gpsimd.tensor_mul(d['gtq'][LAT:2 * LAT, :],
                                     d['g'][LAT:2 * LAT, :],
                                     d['qb'][LAT:2 * LAT, :])

            def s_add_bot(c, d, xt, m2):
                nc.gpsimd.tensor_add(d['Sn'][LAT:2 * LAT, :],
                                     d['Sc'][LAT:2 * LAT, :],
                                     d['gtq'][LAT:2 * LAT, :])

            from concourse.tile import add_dep_helper

            # Cross-step PSUM bank-zero safety (start=True wipes the bank):
            #  - ode1(t+1) zeroes A1: readers tanh_ode(t)/yode(t) are
            #    upstream of add_top(t) which gates ode1(t+1) via Sc.
            #  - g1x_u(t+1) zeroes A2: PE in-order puts it after ns2(t)
            #    >= tanh_ns(t) >= (ACT order) tanh_u(t)/tanh_r(t).
            #  - ns1x(t+1) zeroes B2: emitted after g1s_r(t+1) whose rhs
            #    needs yode(t+1), which is after add_bot(t)/qb(t) in the
            #    DVE queue, covering the n2(t) readers.
            #  - rg2(t+1) zeroes B1: needs tanh_r(t+1), which is after
            #    sig_v(t) in the ACT queue.
            stages = [s_ode1, s_g1x_u, s_g1x_r, s_g1s_r, s_g1s_u,
                      s_tanh_ode, s_g1c_r, s_ode2, s_tanh_r, s_g1c_u,
                      s_yode, s_rg2, s_sig_r, s_tanh_u, s_ns1x, s_ryc,
                      s_ug2, s_ns1s, s_sig_v, s_gm, s_gt1, s_uy,
                      s_tanh_ns, s_ns2, s_abs, s_gtq_top, s_add_top,
                      s_qb, s_gtq_bot, s_add_bot]
            NS = len(stages)
            SIG_IDX = stages.index(s_sig_r)
            OFF = NS // 2
            total = T * NS
            ctx = [None] * CH
            xts = {}
            last_sig = None
            for n in range(total + (CH - 1) * OFF):
                for c in range(CH):
                    m = n - c * OFF
                    if m < 0 or m >= total:
                        continue
                    t, k = divmod(m, NS)
                    if k == 0:
                        if c == 0:
                            xt = xpool.tile([INP, B], bf16, tag="xt",
                                            name=f"xt{t % 4}")
                            nc.sync.dma_start(xt[:, :], xT_d[t])
                            m2 = mpool.tile([INP, B], bf16, tag="m2",
                                            name=f"m2_{t % 4}")
                            nc.gpsimd.dma_start(m2[0:NDATA, :],
                                                xt[NDATA:INP, :])
                            nc.gpsimd.dma_start(m2[NDATA:INP, :],
                                                xt[NDATA:INP, :])
                            xts[t] = (xt, m2)
                        ctx[c] = new_ctx(c, t)
                    stages[k](c, ctx[c], *xts[t])
                    if k == 0 and last_sig is not None:
                        add_dep_helper(ctx[c]['ode1'].ins, last_sig.ins,
                                       sync=False, reason="anti-phase")
                    if k == SIG_IDX:
                        last_sig = ctx[c].get('sig')
                    if k == NS - 1 and c == CH - 1:
                        xts.pop(t)

            # ---- final transform z0 = mlp2([y; s]) ----
            for c in range(CH):
                cs = slice(c * BC, (c + 1) * BC)
                Sf = S[c][T % 2]
                pt1 = bankA1[c][0:TZ, 0:BC]
                nc.tensor.matmul(pt1, wtz1[:, :], Sf[:, :],
                                 start=True, stop=True)
                h_t = tpool[c].tile([TZ, BC], bf16, tag="h_t")
                nc.scalar.activation(h_t[:, :], pt1, ACT.Tanh,
                                     bias=b_act(btz1) if use_bias else 0.0)
                pt2 = bankB1[c][0:2 * LAT, 0:BC]
                nc.tensor.matmul(pt2, wtz2[:, :], h_t[:, :],
                                 start=True, stop=True)
                zo = tpool[c].tile([2 * LAT, BC], f32, tag="zo")
                nc.scalar.activation(zo[0:LAT, :], pt2[0:LAT, :], ACT.Copy,
                                     bias=b_act(btz2t) if use_bias else 0.0)
                nc.scalar.activation(zo[LAT:2 * LAT, :], pt2[LAT:2 * LAT, :],
                                     ACT.Abs,
                                     bias=btz2b[LAT:2 * LAT, :] if use_bias else 0.0)
                nc.sync.dma_start(zout_d[:, cs], zo[:, :])

    nc.compile()
    return nc


def _prep(inputs):
    g = lambda k: np.ascontiguousarray(np.asarray(inputs[k], dtype=np.float32))
    data = g("data")
    tps = g("tps")
    W = {k: g(k) for k in (
        "ug_w1", "ug_b1", "ug_w2", "ug_b2", "rg_w1", "rg_b1", "rg_w2", "rg_b2",
        "ns_w1", "ns_b1", "ns_w2", "ns_b2", "ode_w1", "ode_b1", "ode_w2",
        "ode_b2", "tz_w1", "tz_b1", "tz_w2", "tz_b2")}

    rev = tps[::-1]
    dts = np.concatenate([np.full((1,), -0.01, np.float32),
                          rev[1:] - rev[:-1]]).astype(np.float32)
    dts = tuple(float(d) for d in dts.tolist())

    use_bias = any(float(np.abs(W[k]).max()) != 0.0 for k in W if "_b" in k)

    # time-reverse + transpose: [T, INP, N_TRAJ], contiguous
    xT_full = np.ascontiguousarray(data[:, ::-1, :].transpose(1, 2, 0))

    uniq = list(dict.fromkeys(dts))
    common = {
        "wfuse_ug": np.stack([np.float32(d) * (W["ode_w2"] @ W["ug_w1"][:LAT])
                              for d in uniq]),
        "wfuse_rg": np.stack([np.float32(d) * (W["ode_w2"] @ W["rg_w1"][:LAT])
                              for d in uniq]),
        "wug1": W["ug_w1"],
        "wrg1": W["rg_w1"],
        "wns1": W["ns_w1"],
        "wug2nd": -np.concatenate([W["ug_w2"], W["ug_w2"]], axis=1),
        "wrg2d": np.concatenate([W["rg_w2"], W["rg_w2"]], axis=1),
        "wns2": W["ns_w2"],
        "wode1": W["ode_w1"],
        "wode2": W["ode_w2"],
        "wtz1": W["tz_w1"],
        "wtz2": W["tz_w2"],
        "zeros0": np.zeros((2 * LAT, B), np.float32),
    }
    f32_keys = set()
    if use_bias:
        col = lambda v: np.ascontiguousarray(v.reshape(-1, 1))
        row = lambda v: np.ascontiguousarray(v.reshape(1, -1))
        common.update({
            "bode1": col(W["ode_b1"]),
            "bug1": col(W["ug_b1"]),
            "brg1": col(W["rg_b1"]),
            "bns1": col(W["ns_b1"]),
            "bns2b": col(W["ns_b2"][LAT:]),
            "btz1": col(W["tz_b1"]),
            "btz2t": col(W["tz_b2"][:LAT]),
            "btz2b": col(W["tz_b2"][LAT:]),
            "bug2ndr": row(-np.concatenate([W["ug_b2"], W["ug_b2"]])),
            "brg2dr": row(np.concatenate([W["rg_b2"], W["rg_b2"]])),
            "bns2tr": row(W["ns_b2"][:LAT]),
            "bode2r": row(W["ode_b2"]),
            "ones1": np.ones((1, BC), np.float32),
        })
        f32_keys = {"bode1", "bug1", "brg1", "bns1", "bns2b", "btz1",
                    "btz2t", "btz2b"}

    import ml_dtypes
    bf16 = ml_dtypes.bfloat16
    common = {k: np.ascontiguousarray(
                  v.astype(np.float32 if k in f32_keys else bf16))
              for k, v in common.items()}

    in_maps = []
    for c in range(NCORES):
        m = dict(common)
        m["xT"] = np.ascontiguousarray(
            xT_full[:, :, c * B:(c + 1) * B].astype(bf16))
        in_maps.append(m)
    return in_maps, dts, use_bias


def _ensure_ntff_hook():
    """run_bass_kernel_spmd(trace=True) under axon imports
    antenv.axon_hooks, which is absent in this image. Install a stub so a
    BASS_TRACE=1 environment cannot crash the run."""
    import types as _types
    if "antenv.axon_hooks" in sys.modules:
        return
    hook = None
    try:
        from trn_agent_boot.trn_boot import _ntff_profile_via_ctypes
        hook = _ntff_profile_via_ctypes("/opt/axon/libaxon_pjrt.so")
    except Exception:
        hook = None
    try:
        import antenv
        mod = _types.ModuleType("antenv.axon_hooks")
        mod.get_axon_ntff_profile_hook = lambda: hook
        mod.set_axon_ntff_profile_hook = lambda h: None
        sys.modules["antenv.axon_hooks"] = mod
        antenv.axon_hooks = mod
    except Exception:
        pass


def _run(inputs, trace=False, trace_kwargs=None):
    _ensure_ntff_hook()
    from concourse.bass_utils import run_bass_kernel_spmd

    in_maps, dts, use_bias = _prep(inputs)
    key = (dts, use_bias)
    if key not in _cache:
        _cache[key] = _build(dts, use_bias)
    nc = _cache[key]

    res = run_bass_kernel_spmd(nc, in_maps, list(range(NCORES)),
                               trace=trace, **(trace_kwargs or {}))
    mu = np.empty((N_TRAJ, LAT), np.float32)
    std = np.empty((N_TRAJ, LAT), np.float32)
    for c in range(NCORES):
        z = res.results[c]["zout"]
        mu[c * B:(c + 1) * B] = z[0:LAT].T
        std[c * B:(c + 1) * B] = z[LAT:2 * LAT].T
    return (mu[None], std[None]), res


def kernel(**inputs):
    out, _ = _run(inputs, trace=False)
    return out


# revision 31
# speedup vs baseline: 1.2431x; 1.2431x over previous
"""ODE-RNN encoder (GRU-ODE scan) Trainium2 Bass kernel.

Strategy (data-parallel over trajectories):
  - 4096 trajectories sharded 512/core over 8 NeuronCores; all weights
    replicated. The T=128 time scan runs locally per core, no cross-core
    communication. Host gathers the per-core z0 outputs at the end.
  - On-chip layout is feature-on-partition, batch-on-free-dim, bf16
    everywhere except PSUM accumulation. Each core's 512-batch is split
    into 2 dephased chunks of 256 so the serial per-step dependency chain
    of one chunk hides under engine work of the other.
  - The wall clock is 128 x the single-step chain latency, so the design
    minimizes chain hops: 11 matmuls per chunk-step (no ODE-correction
    matmuls, no -I matmul), gates read the post-ODE state written in
    place by one DVE op, and the blend top half is
      y' = (y_ode - g*y_ode) + g*ns_t
    with the (1-g)*y_ode part computed off-chain. abs(ns_b) runs on DVE
    (max(x,-x)) to keep the ACT engine at 6 ops/chunk-step.
  - PSUM start=True zeroes the ENTIRE 2 KiB bank (hardware zero region),
    so each bank runs ONE accumulation group per step: first matmul
    carries start, last carries stop, middles accumulate. Banks per
    chunk: A1=[ode hidden|ode out], A2=[gate1 u|r], B1=[gate2 u|r],
    B2=[ns hidden|ns out]. Cross-step zeroing safety is by engine
    queue order (see stage list comments).

kernel(**inputs) takes the full unsharded numpy inputs (as produced by the
reference setup) and returns (z0_mu, z0_std), each (1, 4096, 64) float32.
"""

import os
import sys

import numpy as np

N_TRAJ = 4096
T = 128
LAT = 64
NDATA = 64
INP = 2 * NDATA
NGRU = 100
NODE = 100
TZ = 100
NCORES = 8
B = N_TRAJ // NCORES          # 512 per core
CH = 2                        # chunks per core
BC = B // CH                  # 256 batch per chunk

_cache = {}


def _build(dts, use_bias):
    import concourse.bass as bass
    import concourse.tile as tile
    from concourse import bacc, mybir

    uniq = list(dict.fromkeys(dts))
    dt_idx = [uniq.index(d) for d in dts]
    n_dt = len(uniq)

    f32 = mybir.dt.float32
    bf16 = mybir.dt.bfloat16
    ACT = mybir.ActivationFunctionType
    ALU = mybir.AluOpType

    nc = bacc.Bacc("TRN2", target_bir_lowering=False, debug=False,
                   num_devices=NCORES)

    # ---- DRAM I/O ----
    xT_d = nc.dram_tensor("xT", [T, INP, B], bf16, kind="ExternalInput")
    wug1_d = nc.dram_tensor("wug1", [2 * LAT + INP, NGRU], bf16, kind="ExternalInput")
    wrg1_d = nc.dram_tensor("wrg1", [2 * LAT + INP, NGRU], bf16, kind="ExternalInput")
    wns1_d = nc.dram_tensor("wns1", [2 * LAT + INP, NGRU], bf16, kind="ExternalInput")
    wug2_d = nc.dram_tensor("wug2nd", [NGRU, 2 * LAT], bf16, kind="ExternalInput")
    wrg2_d = nc.dram_tensor("wrg2d", [NGRU, 2 * LAT], bf16, kind="ExternalInput")
    wns2_d = nc.dram_tensor("wns2", [NGRU, 2 * LAT], bf16, kind="ExternalInput")
    wode1_d = nc.dram_tensor("wode1", [LAT, NODE], bf16, kind="ExternalInput")
    wode2_d = nc.dram_tensor("wode2", [NODE, LAT], bf16, kind="ExternalInput")
    wfug_d = nc.dram_tensor("wfuse_ug", [n_dt, NODE, NGRU], bf16, kind="ExternalInput")
    wfrg_d = nc.dram_tensor("wfuse_rg", [n_dt, NODE, NGRU], bf16, kind="ExternalInput")
    wtz1_d = nc.dram_tensor("wtz1", [2 * LAT, TZ], bf16, kind="ExternalInput")
    wtz2_d = nc.dram_tensor("wtz2", [TZ, 2 * LAT], bf16, kind="ExternalInput")
    if use_bias:
        bode1_d = nc.dram_tensor("bode1", [NODE, 1], f32, kind="ExternalInput")
        bug1_d = nc.dram_tensor("bug1", [NGRU, 1], f32, kind="ExternalInput")
        brg1_d = nc.dram_tensor("brg1", [NGRU, 1], f32, kind="ExternalInput")
        bns1_d = nc.dram_tensor("bns1", [NGRU, 1], f32, kind="ExternalInput")
        bns2b_d = nc.dram_tensor("bns2b", [LAT, 1], f32, kind="ExternalInput")
        btz1_d = nc.dram_tensor("btz1", [TZ, 1], f32, kind="ExternalInput")
        btz2t_d = nc.dram_tensor("btz2t", [LAT, 1], f32, kind="ExternalInput")
        btz2b_d = nc.dram_tensor("btz2b", [LAT, 1], f32, kind="ExternalInput")
        # row-vector biases (K=1 matmul accumulate): [1, M]
        bug2_d = nc.dram_tensor("bug2ndr", [1, 2 * LAT], bf16, kind="ExternalInput")
        brg2_d = nc.dram_tensor("brg2dr", [1, 2 * LAT], bf16, kind="ExternalInput")
        bns2t_d = nc.dram_tensor("bns2tr", [1, LAT], bf16, kind="ExternalInput")
        bode2_d = nc.dram_tensor("bode2r", [1, LAT], bf16, kind="ExternalInput")
        ones_d = nc.dram_tensor("ones1", [1, BC], bf16, kind="ExternalInput")
    zeros_d = nc.dram_tensor("zeros0", [2 * LAT, B], bf16, kind="ExternalInput")
    zout_d = nc.dram_tensor("zout", [2 * LAT, B], f32, kind="ExternalOutput")

    with tile.TileContext(nc) as tc:
        with (
            tc.tile_pool(name="const", bufs=1) as cpool,
            tc.tile_pool(name="state", bufs=1) as spool,
            tc.tile_pool(name="xin", bufs=3) as xpool,
            tc.tile_pool(name="mdup", bufs=2) as mpool,
            tc.tile_pool(name="tmp0", bufs=2) as tpool0,
            tc.tile_pool(name="tmp1", bufs=2) as tpool1,
            tc.tile_pool(name="bkA10", bufs=1, space="PSUM") as bA10,
            tc.tile_pool(name="bkA20", bufs=1, space="PSUM") as bA20,
            tc.tile_pool(name="bkB10", bufs=1, space="PSUM") as bB10,
            tc.tile_pool(name="bkB20", bufs=1, space="PSUM") as bB20,
            tc.tile_pool(name="bkA11", bufs=1, space="PSUM") as bA11,
            tc.tile_pool(name="bkA21", bufs=1, space="PSUM") as bA21,
            tc.tile_pool(name="bkB11", bufs=1, space="PSUM") as bB11,
            tc.tile_pool(name="bkB21", bufs=1, space="PSUM") as bB21,
        ):
            tpool = [tpool0, tpool1]
            bA1p = [bA10, bA11]
            bA2p = [bA20, bA21]
            bB1p = [bB10, bB11]
            bB2p = [bB20, bB21]

            # ---- load constants ----
            def cload(shape, src_ap, tag, dt_=None):
                t = cpool.tile(shape, dt_ or bf16, tag=tag, name=tag)
                nc.sync.dma_start(t[:, :], src_ap)
                return t

            wug1a = cload([INP, NGRU], wug1_d[0:INP, :], "wug1a")
            wug1b = cload([INP, NGRU], wug1_d[INP:2 * LAT + INP, :], "wug1b")
            wrg1a = cload([INP, NGRU], wrg1_d[0:INP, :], "wrg1a")
            wrg1b = cload([INP, NGRU], wrg1_d[INP:2 * LAT + INP, :], "wrg1b")
            wns1a = cload([INP, NGRU], wns1_d[0:INP, :], "wns1a")
            wns1b = cload([INP, NGRU], wns1_d[INP:2 * LAT + INP, :], "wns1b")
            # rows 0:128 of w*1 multiply [y;s] (=128 rows), rows 128:256
            # multiply x (=128 rows); INP == 2*LAT == 128 here.
            wug2 = cload([NGRU, 2 * LAT], wug2_d[:, :], "wug2")
            wrg2 = cload([NGRU, 2 * LAT], wrg2_d[:, :], "wrg2")
            wns2 = cload([NGRU, 2 * LAT], wns2_d[:, :], "wns2")
            wode1 = cload([LAT, NODE], wode1_d[:, :], "wode1")
            wode2 = cload([NODE, LAT], wode2_d[:, :], "wode2")
            wfug = [cload([NODE, NGRU], wfug_d[i], f"wfug{i}")
                    for i in range(n_dt)]
            wfrg = [cload([NODE, NGRU], wfrg_d[i], f"wfrg{i}")
                    for i in range(n_dt)]
            wtz1 = cload([2 * LAT, TZ], wtz1_d[:, :], "wtz1")
            wtz2 = cload([TZ, 2 * LAT], wtz2_d[:, :], "wtz2")
            if use_bias:
                bode1 = cload([NODE, 1], bode1_d[:, :], "bode1", f32)
                bug1 = cload([NGRU, 1], bug1_d[:, :], "bug1", f32)
                brg1 = cload([NGRU, 1], brg1_d[:, :], "brg1", f32)
                bns1 = cload([NGRU, 1], bns1_d[:, :], "bns1", f32)
                btz1 = cload([TZ, 1], btz1_d[:, :], "btz1", f32)
                btz2t = cload([LAT, 1], btz2t_d[:, :], "btz2t", f32)
                bns2b = cpool.tile([2 * LAT, 1], f32, tag="bns2b", name="bns2b")
                nc.sync.dma_start(bns2b[LAT:2 * LAT, :], bns2b_d[:, :])
                btz2b = cpool.tile([2 * LAT, 1], f32, tag="btz2b", name="btz2b")
                nc.sync.dma_start(btz2b[LAT:2 * LAT, :], btz2b_d[:, :])
                bug2r = cload([1, 2 * LAT], bug2_d[:, :], "bug2r")
                brg2r = cload([1, 2 * LAT], brg2_d[:, :], "brg2r")
                bns2tr = cload([1, LAT], bns2t_d[:, :], "bns2tr")
                bode2r = cload([1, LAT], bode2_d[:, :], "bode2r")
                ones = cpool.tile([1, BC], bf16, tag="ones", name="ones")
                nc.sync.dma_start(ones[:, :], ones_d[:, :])

            def b_act(t):
                return t[:, :] if use_bias else 0.0

            # ---- state tiles (ping-pong per chunk) ----
            S = [[spool.tile([2 * LAT, BC], bf16, tag=f"s{c}_{p}",
                             name=f"s{c}_{p}")
                  for p in range(2)] for c in range(CH)]
            for c in range(CH):
                nc.sync.dma_start(S[c][0][:, :],
                                  zeros_d[:, c * BC:(c + 1) * BC])

            # ---- PSUM banks: 4 per chunk, 8 total (each [128, 2*BC] f32
            # = 2 KiB/partition = exactly one bank) ----
            bankA1 = [bA1p[c].tile([128, 2 * BC], f32, tag="bA1",
                                   name=f"bA1_{c}") for c in range(CH)]
            bankA2 = [bA2p[c].tile([128, 2 * BC], f32, tag="bA2",
                                   name=f"bA2_{c}") for c in range(CH)]
            bankB1 = [bB1p[c].tile([128, 2 * BC], f32, tag="bB1",
                                   name=f"bB1_{c}") for c in range(CH)]
            bankB2 = [bB2p[c].tile([128, 2 * BC], f32, tag="bB2",
                                   name=f"bB2_{c}") for c in range(CH)]

            def new_ctx(c, t):
                return dict(cs=slice(c * BC, (c + 1) * BC),
                            Sc=S[c][t % 2], Sn=S[c][(t + 1) % 2],
                            tp=tpool[c], t=t,
                            A1=bankA1[c], A2=bankA2[c],
                            B1=bankB1[c], B2=bankB2[c])

            # --- stages, emission order == chain order ---
            def s_ode1(c, d, xt, m2):
                d['oh'] = d['A1'][0:NODE, 0:BC]
                d['ode1'] = nc.tensor.matmul(d['oh'], wode1[:, :],
                                             d['Sc'][0:LAT, :],
                                             start=True, stop=False)

            def s_g1x_u(c, d, xt, m2):
                d['g1'] = d['A2'][0:NGRU, 0:2 * BC]
                nc.tensor.matmul(d['g1'][:, 0:BC], wug1b[:, :],
                                 xt[:, d['cs']], start=True, stop=False)

            def s_g1x_r(c, d, xt, m2):
                nc.tensor.matmul(d['g1'][:, BC:2 * BC], wrg1b[:, :],
                                 xt[:, d['cs']], start=False, stop=False,
                                 skip_group_check=True)

            def s_tanh_ode(c, d, xt, m2):
                d['h_ode'] = d['tp'].tile([NODE, BC], bf16, tag="h_ode",
                                          name=f"ho{c}")
                nc.scalar.activation(d['h_ode'][:, :], d['oh'], ACT.Tanh,
                                     bias=b_act(bode1) if use_bias else 0.0)

            def s_ode2(c, d, xt, m2):
                d['yo'] = d['A1'][0:LAT, BC:2 * BC]
                nc.tensor.matmul(d['yo'], wode2[:, :], d['h_ode'][:, :],
                                 start=False, stop=not use_bias,
                                 skip_group_check=True)
                if use_bias:
                    nc.tensor.matmul(d['yo'], bode2r[:, :], ones[:, :],
                                     start=False, stop=True,
                                     skip_group_check=True)

            def s_yode(c, d, xt, m2):
                # in-place: Sc[0:LAT] <- y + dt * ode_out
                nc.vector.scalar_tensor_tensor(
                    d['Sc'][0:LAT, :], d['yo'], float(dts[d['t']]),
                    d['Sc'][0:LAT, :], op0=ALU.mult, op1=ALU.add)

            # gate-1 state matmuls read the PRE-ODE state (emitted before
            # yode's in-place write); the fused dt*(ode_w2 @ W_g1[:64])
            # correction matmuls bring them to the post-ODE value without
            # waiting on the yode DVE hop.
            def s_g1s_r(c, d, xt, m2):
                nc.tensor.matmul(d['g1'][:, BC:2 * BC], wrg1a[:, :],
                                 d['Sc'][:, :], start=False, stop=False,
                                 skip_group_check=True)

            def s_g1c_r(c, d, xt, m2):
                nc.tensor.matmul(d['g1'][:, BC:2 * BC],
                                 wfrg[dt_idx[d['t']]][:, :],
                                 d['h_ode'][:, :], start=False, stop=False,
                                 skip_group_check=True)

            def s_g1c_u(c, d, xt, m2):
                nc.tensor.matmul(d['g1'][:, 0:BC],
                                 wfug[dt_idx[d['t']]][:, :],
                                 d['h_ode'][:, :], start=False, stop=True,
                                 skip_group_check=True)

            def s_tanh_r(c, d, xt, m2):
                d['h_g'] = d['tp'].tile([NGRU, 2 * BC], bf16, tag="h_g",
                                        name=f"hg{c}")
                nc.scalar.activation(d['h_g'][:, BC:2 * BC],
                                     d['g1'][:, BC:2 * BC], ACT.Tanh,
                                     bias=b_act(brg1) if use_bias else 0.0)

            def s_ns1x(c, d, xt, m2):
                d['n1'] = d['B2'][0:NGRU, 0:BC]
                nc.tensor.matmul(d['n1'], wns1b[:, :],
                                 xt[:, d['cs']], start=True, stop=False)

            def s_g1s_u(c, d, xt, m2):
                nc.tensor.matmul(d['g1'][:, 0:BC], wug1a[:, :],
                                 d['Sc'][:, :], start=False, stop=False,
                                 skip_group_check=True)

            def s_rg2(c, d, xt, m2):
                d['g2'] = d['B1'][0:2 * LAT, 0:2 * BC]
                nc.tensor.matmul(d['g2'][:, BC:2 * BC], wrg2[:, :],
                                 d['h_g'][:, BC:2 * BC],
                                 start=True, stop=False)
                if use_bias:
                    nc.tensor.matmul(d['g2'][:, BC:2 * BC], brg2r[:, :],
                                     ones[:, :], start=False, stop=False,
                                     skip_group_check=True)

            def s_sig_r(c, d, xt, m2):
                d['vr'] = d['tp'].tile([2 * LAT, 2 * BC], bf16, tag="vr",
                                       name=f"vr{c}")
                d['sig'] = nc.scalar.activation(d['vr'][:, BC:2 * BC],
                                                d['g2'][:, BC:2 * BC],
                                                ACT.Sigmoid)

            def s_tanh_u(c, d, xt, m2):
                nc.scalar.activation(d['h_g'][:, 0:BC], d['g1'][:, 0:BC],
                                     ACT.Tanh,
                                     bias=b_act(bug1) if use_bias else 0.0)

            def s_ryc(c, d, xt, m2):
                d['ryc'] = d['tp'].tile([2 * LAT, BC], bf16, tag="ryc",
                                        name=f"ryc{c}")
                nc.vector.tensor_mul(d['ryc'][:, :], d['vr'][:, BC:2 * BC],
                                     d['Sc'][:, :])

            def s_ug2(c, d, xt, m2):
                nc.tensor.matmul(d['g2'][:, 0:BC], wug2[:, :],
                                 d['h_g'][:, 0:BC],
                                 start=False, stop=not use_bias,
                                 skip_group_check=True)
                if use_bias:
                    nc.tensor.matmul(d['g2'][:, 0:BC], bug2r[:, :],
                                     ones[:, :], start=False, stop=True,
                                     skip_group_check=True)

            def s_ns1s(c, d, xt, m2):
                nc.tensor.matmul(d['n1'], wns1a[:, :],
                                 d['ryc'][:, :], start=False, stop=False,
                                 skip_group_check=True)

            def s_sig_v(c, d, xt, m2):
                nc.scalar.activation(d['vr'][:, 0:BC], d['g2'][:, 0:BC],
                                     ACT.Sigmoid)

            def s_gm(c, d, xt, m2):
                d['g'] = d['tp'].tile([2 * LAT, BC], bf16, tag="g",
                                      name=f"g{c}")
                nc.vector.tensor_mul(d['g'][:, :], m2[:, d['cs']],
                                     d['vr'][:, 0:BC])

            def s_gt1(c, d, xt, m2):
                # t1 = g_top * y_ode (feeds add_top, keep on fast DVE)
                d['t1'] = d['tp'].tile([LAT, BC], bf16, tag="t1",
                                       name=f"t1{c}")
                nc.vector.tensor_mul(d['t1'][:, :], d['g'][0:LAT, :],
                                     d['Sc'][0:LAT, :])

            def s_uy(c, d, xt, m2):
                # uy = (1 - g_top) * y_ode
                d['uy'] = d['tp'].tile([LAT, BC], bf16, tag="uy",
                                       name=f"uy{c}")
                nc.vector.tensor_sub(d['uy'][:, :], d['Sc'][0:LAT, :],
                                     d['t1'][:, :])

            def s_tanh_ns(c, d, xt, m2):
                d['h_n'] = d['tp'].tile([NGRU, BC], bf16, tag="h_n",
                                        name=f"hn{c}")
                nc.scalar.activation(d['h_n'][:, :], d['n1'], ACT.Tanh,
                                     bias=b_act(bns1) if use_bias else 0.0)

            def s_ns2(c, d, xt, m2):
                d['n2'] = d['B2'][0:2 * LAT, BC:2 * BC]
                nc.tensor.matmul(d['n2'], wns2[:, :], d['h_n'][:, :],
                                 start=False, stop=not use_bias,
                                 skip_group_check=True)
                if use_bias:
                    nc.tensor.matmul(d['n2'][0:LAT, :], bns2tr[:, :],
                                     ones[:, :], start=False, stop=True,
                                     skip_group_check=True)

            def s_abs(c, d, xt, m2):
                # |ns_b| on DVE: max(x, -x); ACT path only when a bias
                # must be added before the abs.
                d['absb'] = d['tp'].tile([2 * LAT, BC], bf16, tag="absb",
                                         name=f"ab{c}")
                nc.scalar.activation(d['absb'][LAT:2 * LAT, :],
                                     d['n2'][LAT:2 * LAT, :], ACT.Abs,
                                     bias=bns2b[LAT:2 * LAT, :] if use_bias
                                     else 0.0)

            def s_gtq_top(c, d, xt, m2):
                d['gtq'] = d['tp'].tile([2 * LAT, BC], bf16, tag="gtq",
                                        name=f"gtq{c}")
                nc.vector.tensor_mul(d['gtq'][0:LAT, :], d['g'][0:LAT, :],
                                     d['n2'][0:LAT, :])

            def s_add_top(c, d, xt, m2):
                nc.vector.tensor_add(d['Sn'][0:LAT, :], d['uy'][:, :],
                                     d['gtq'][0:LAT, :])

            # bottom (std) half of the blend is pure slack: its result is
            # first needed by g1s of the NEXT step, ~a full half-period
            # later, so it runs on the otherwise-idle GPSIMD engine.
            def s_qb(c, d, xt, m2):
                d['qb'] = d['tp'].tile([2 * LAT, BC], bf16, tag="qb",
                                       name=f"qb{c}")
                nc.vector.tensor_sub(d['qb'][LAT:2 * LAT, :],
                                     d['absb'][LAT:2 * LAT, :],
                                     d['Sc'][LAT:2 * LAT, :])

            def s_gtq_bot(c, d, xt, m2):
                nc.vector.tensor_mul(d['gtq'][LAT:2 * LAT, :],
                                     d['g'][LAT:2 * LAT, :],
                                     d['qb'][LAT:2 * LAT, :])

            def s_add_bot(c, d, xt, m2):
                nc.vector.tensor_add(d['Sn'][LAT:2 * LAT, :],
                                     d['Sc'][LAT:2 * LAT, :],
                                     d['gtq'][LAT:2 * LAT, :])

            from concourse.tile import add_dep_helper

            # Cross-step PSUM bank-zero safety (start=True wipes the bank):
            #  - ode1(t+1) zeroes A1: readers tanh_ode(t)/yode(t) are
            #    upstream of add_top(t) which gates ode1(t+1) via Sc.
            #  - g1x_u(t+1) zeroes A2: PE in-order puts it after ns2(t)
            #    >= tanh_ns(t) >= (ACT order) tanh_u(t)/tanh_r(t).
            #  - ns1x(t+1) zeroes B2: emitted after g1s_r(t+1) whose rhs
            #    needs yode(t+1), which is after add_bot(t)/qb(t) in the
            #    DVE queue, covering the n2(t) readers.
            #  - rg2(t+1) zeroes B1: needs tanh_r(t+1), which is after
            #    sig_v(t) in the ACT queue.
            stages = [s_ode1, s_g1x_u, s_g1x_r, s_g1s_r, s_g1s_u,
                      s_tanh_ode, s_g1c_r, s_ode2, s_tanh_r, s_g1c_u,
                      s_yode, s_rg2, s_sig_r, s_tanh_u, s_ns1x, s_ryc,
                      s_ug2, s_ns1s, s_sig_v, s_gm, s_gt1, s_uy,
                      s_tanh_ns, s_ns2, s_abs, s_gtq_top, s_add_top,
                      s_qb, s_gtq_bot, s_add_bot]
            NS = len(stages)
            SIG_IDX = stages.index(s_sig_r)
            OFF = NS // 2
            total = T * NS
            ctx = [None] * CH
            xts = {}
            last_sig = None
            for n in range(total + (CH - 1) * OFF):
                for c in range(CH):
                    m = n - c * OFF
                    if m < 0 or m >= total:
                        continue
                    t, k = divmod(m, NS)
                    if k == 0:
                        if c == 0:
                            xt = xpool.tile([INP, B], bf16, tag="xt",
                                            name=f"xt{t % 4}")
                            nc.sync.dma_start(xt[:, :], xT_d[t])
                            m2 = mpool.tile([INP, B], bf16, tag="m2",
                                            name=f"m2_{t % 4}")
                            nc.gpsimd.dma_start(m2[0:NDATA, :],
                                                xt[NDATA:INP, :])
                            nc.gpsimd.dma_start(m2[NDATA:INP, :],
                                                xt[NDATA:INP, :])
                            xts[t] = (xt, m2)
                        ctx[c] = new_ctx(c, t)
                    stages[k](c, ctx[c], *xts[t])
                    if k == 0 and last_sig is not None:
                        add_dep_helper(ctx[c]['ode1'].ins, last_sig.ins,
                                       sync=False, reason="anti-phase")
                    if k == SIG_IDX:
                        last_sig = ctx[c].get('sig')
                    if k == NS - 1 and c == CH - 1:
                        xts.pop(t)

            # ---- final transform z0 = mlp2([y; s]) ----
            for c in range(CH):
                cs = slice(c * BC, (c + 1) * BC)
                Sf = S[c][T % 2]
                pt1 = bankA1[c][0:TZ, 0:BC]
                nc.tensor.matmul(pt1, wtz1[:, :], Sf[:, :],
                                 start=True, stop=True)
                h_t = tpool[c].tile([TZ, BC], bf16, tag="h_t")
                nc.scalar.activation(h_t[:, :], pt1, ACT.Tanh,
                                     bias=b_act(btz1) if use_bias else 0.0)
                pt2 = bankB1[c][0:2 * LAT, 0:BC]
                nc.tensor.matmul(pt2, wtz2[:, :], h_t[:, :],
                                 start=True, stop=True)
                zo = tpool[c].tile([2 * LAT, BC], f32, tag="zo")
                nc.scalar.activation(zo[0:LAT, :], pt2[0:LAT, :], ACT.Copy,
                                     bias=b_act(btz2t) if use_bias else 0.0)
                nc.scalar.activation(zo[LAT:2 * LAT, :], pt2[LAT:2 * LAT, :],
                                     ACT.Abs,
                                     bias=btz2b[LAT:2 * LAT, :] if use_bias else 0.0)
                nc.sync.dma_start(zout_d[:, cs], zo[:, :])

    nc.compile()
    return nc


def _prep(inputs):
    g = lambda k: np.ascontiguousarray(np.asarray(inputs[k], dtype=np.float32))
    data = g("data")
    tps = g("tps")
    W = {k: g(k) for k in (
        "ug_w1", "ug_b1", "ug_w2", "ug_b2", "rg_w1", "rg_b1", "rg_w2", "rg_b2",
        "ns_w1", "ns_b1", "ns_w2", "ns_b2", "ode_w1", "ode_b1", "ode_w2",
        "ode_b2", "tz_w1", "tz_b1", "tz_w2", "tz_b2")}

    rev = tps[::-1]
    dts = np.concatenate([np.full((1,), -0.01, np.float32),
                          rev[1:] - rev[:-1]]).astype(np.float32)
    dts = tuple(float(d) for d in dts.tolist())

    use_bias = any(float(np.abs(W[k]).max()) != 0.0 for k in W if "_b" in k)

    # time-reverse + transpose: [T, INP, N_TRAJ], contiguous
    xT_full = np.ascontiguousarray(data[:, ::-1, :].transpose(1, 2, 0))

    uniq = list(dict.fromkeys(dts))
    common = {
        "wfuse_ug": np.stack([np.float32(d) * (W["ode_w2"] @ W["ug_w1"][:LAT])
                              for d in uniq]),
        "wfuse_rg": np.stack([np.float32(d) * (W["ode_w2"] @ W["rg_w1"][:LAT])
                              for d in uniq]),
        "wug1": W["ug_w1"],
        "wrg1": W["rg_w1"],
        "wns1": W["ns_w1"],
        "wug2nd": -np.concatenate([W["ug_w2"], W["ug_w2"]], axis=1),
        "wrg2d": np.concatenate([W["rg_w2"], W["rg_w2"]], axis=1),
        "wns2": W["ns_w2"],
        "wode1": W["ode_w1"],
        "wode2": W["ode_w2"],
        "wtz1": W["tz_w1"],
        "wtz2": W["tz_w2"],
        "zeros0": np.zeros((2 * LAT, B), np.float32),
    }
    f32_keys = set()
    if use_bias:
        col = lambda v: np.ascontiguousarray(v.reshape(-1, 1))
        row = lambda v: np.ascontiguousarray(v.reshape(1, -1))
        common.update({
            "bode1": col(W["ode_b1"]),
            "bug1": col(W["ug_b1"]),
            "brg1": col(W["rg_b1"]),
            "bns1": col(W["ns_b1"]),
            "bns2b": col(W["ns_b2"][LAT:]),
            "btz1": col(W["tz_b1"]),
            "btz2t": col(W["tz_b2"][:LAT]),
            "btz2b": col(W["tz_b2"][LAT:]),
            "bug2ndr": row(-np.concatenate([W["ug_b2"], W["ug_b2"]])),
            "brg2dr": row(np.concatenate([W["rg_b2"], W["rg_b2"]])),
            "bns2tr": row(W["ns_b2"][:LAT]),
            "bode2r": row(W["ode_b2"]),
            "ones1": np.ones((1, BC), np.float32),
        })
        f32_keys = {"bode1", "bug1", "brg1", "bns1", "bns2b", "btz1",
                    "btz2t", "btz2b"}

    import ml_dtypes
    bf16 = ml_dtypes.bfloat16
    common = {k: np.ascontiguousarray(
                  v.astype(np.float32 if k in f32_keys else bf16))
              for k, v in common.items()}

    in_maps = []
    for c in range(NCORES):
        m = dict(common)
        m["xT"] = np.ascontiguousarray(
            xT_full[:, :, c * B:(c + 1) * B].astype(bf16))
        in_maps.append(m)
    return in_maps, dts, use_bias


def _ensure_ntff_hook():
    """run_bass_kernel_spmd(trace=True) under axon imports
    antenv.axon_hooks, which is absent in this image. Install a stub so a
    BASS_TRACE=1 environment cannot crash the run."""
    import types as _types
    if "antenv.axon_hooks" in sys.modules:
        return
    hook = None
    try:
        from trn_agent_boot.trn_boot import _ntff_profile_via_ctypes
        hook = _ntff_profile_via_ctypes("/opt/axon/libaxon_pjrt.so")
    except Exception:
        hook = None
    try:
        import antenv
        mod = _types.ModuleType("antenv.axon_hooks")
        mod.get_axon_ntff_profile_hook = lambda: hook
        mod.set_axon_ntff_profile_hook = lambda h: None
        sys.modules["antenv.axon_hooks"] = mod
        antenv.axon_hooks = mod
    except Exception:
        pass


def _run(inputs, trace=False, trace_kwargs=None):
    _ensure_ntff_hook()
    from concourse.bass_utils import run_bass_kernel_spmd

    in_maps, dts, use_bias = _prep(inputs)
    key = (dts, use_bias)
    if key not in _cache:
        _cache[key] = _build(dts, use_bias)
    nc = _cache[key]

    res = run_bass_kernel_spmd(nc, in_maps, list(range(NCORES)),
                               trace=trace, **(trace_kwargs or {}))
    mu = np.empty((N_TRAJ, LAT), np.float32)
    std = np.empty((N_TRAJ, LAT), np.float32)
    for c in range(NCORES):
        z = res.results[c]["zout"]
        mu[c * B:(c + 1) * B] = z[0:LAT].T
        std[c * B:(c + 1) * B] = z[LAT:2 * LAT].T
    return (mu[None], std[None]), res


def kernel(**inputs):
    out, _ = _run(inputs, trace=False)
    return out
